# revision 24
# baseline (speedup 1.0000x reference)
"""DBRX MoE experts kernel for Trainium2 (8 NeuronCores).

Strategy (expert-parallel + TP-8 overflow tail, bf16):
  - Router (logits -> softmax -> top-2 -> renormalize) on host in numpy
    (0.01% of FLOPs); it determines the token->expert dispatch.
  - Main phase, expert-parallel: core c owns expert c (counts are near-
    balanced, ~1024 +- 40). Each core runs the full SwiGLU FFN for the first
    `base`=1024 tokens of its expert, scaling rows by the combine weight.
    No collective: the host adds the two expert contributions per token.
  - Overflow tail, tensor-parallel: the few tokens above `base` per expert
    (~90 total) are processed by ALL cores, each holding the I/8 shard of the
    run's expert weights; the host sums the 8 partial outputs. This removes
    the per-core padding to the max expert count: every core does identical
    work with zero load imbalance.
  - All matmuls bf16 (fp32 PSUM): full PE rate at any free dim, half the DMA
    and SBUF of fp32r. Main weights stream once per 512-token macro-pass
    (2 passes); x and h stay SBUF-resident. Tail runs' weight streams and
    compute are interleaved into the down-proj phases' DMA/PE slack.
"""

import numpy as np
import ml_dtypes

T = 4096
D = 2048
E = 8
I = 4096
TOPK = 2
NCORES = 8
P = 128
DCH = D // P  # 16 d-chunks (contraction tiles for gate/up)
IG = I // 256  # 16 i-groups of 256 (gate/up psum-pair granularity)
ICH = I // P  # 32 i-chunks (contraction tiles for down)
DQ = D // 512  # 4 output-column quarters for down
ISH = I // NCORES  # 512, tail i-shard

BF16 = ml_dtypes.bfloat16


def _host_router(x, router_w):
    """Replicate reference routing in numpy (fp32)."""
    logits = (x.astype(np.float64) @ router_w.astype(np.float64).T).astype(np.float32)
    m = logits.max(axis=-1, keepdims=True)
    ex = np.exp((logits - m).astype(np.float32))
    probs = ex / ex.sum(axis=-1, keepdims=True)
    # top-2, ties to lower index (matches jax.lax.top_k)
    top1 = probs.argmax(axis=-1)
    p = probs.copy()
    p[np.arange(T), top1] = -1.0
    top2 = p.argmax(axis=-1)
    w1 = probs[np.arange(T), top1]
    w2 = probs[np.arange(T), top2]
    s = w1 + w2
    return top1.astype(np.int64), top2.astype(np.int64), (w1 / s).astype(np.float32), (w2 / s).astype(np.float32)


def _ranges(t0, n, step):
    out = []
    t = t0
    while t0 + n - t > 0:
        sz = min(step, t0 + n - t)
        out.append((t, sz))
        t += sz
    return out


def _passes(cap):
    """Split [0, cap) into macro-passes of <=512 tokens (single gate/up psum
    group per pass)."""
    passes = []
    for t0, n in _ranges(0, cap, 512):
        groups = _ranges(t0, n, 512)
        subs = _ranges(t0, n, 128)
        passes.append((groups, subs))
    return passes


_CACHE: dict = {}


def _build_bass(base: int, run_sizes: tuple):
    """8-core SPMD program. Main: `base` token slots, expert-parallel.
    Tail: the given runs (<=128 tokens each), TP-8 over the I dim, partial
    outputs per core (host sums across cores)."""
    import concourse.bacc as bacc
    import concourse.mybir as mybir
    import concourse.tile as tile

    f32 = mybir.dt.float32
    bf16 = mybir.dt.bfloat16
    passes = _passes(base)
    nsub = sum(len(subs) for _, subs in passes)
    max_pass = max(g[-1][0] + g[-1][1] - g[0][0] for g, _ in passes)
    IQ = ICH // 4  # 8 i-chunks per main w2 quarter tile
    nruns = len(run_sizes)
    TT = sum(run_sizes)
    run_off = [sum(run_sizes[:r]) for r in range(nruns)]

    nc = bacc.Bacc("TRN2", target_bir_lowering=False)

    xtp_d = nc.dram_tensor("xtp", [P, DCH, base], bf16, kind="ExternalInput")
    wst_d = nc.dram_tensor("wst", [IG, P, DCH, 512], bf16, kind="ExternalInput")
    w2t_d = nc.dram_tensor("w2t", [DQ, P, ICH, 512], bf16, kind="ExternalInput")
    cw_d = nc.dram_tensor("cw", [P, nsub], f32, kind="ExternalInput")
    out_d = nc.dram_tensor("out", [base, D], f32, kind="ExternalOutput")
    if nruns:
        xt_d = nc.dram_tensor("xt_t", [P, DCH, TT], bf16, kind="ExternalInput")
        wst_t_d = nc.dram_tensor("wst_t", [nruns, 2, P, DCH, 512], bf16, kind="ExternalInput")
        w2t_t_d = nc.dram_tensor("w2t_t", [nruns, 2, P, 4, 1024], bf16, kind="ExternalInput")
        cwt_d = nc.dram_tensor("cwt", [P, nruns], f32, kind="ExternalInput")
        tout_d = nc.dram_tensor("tout", [TT, D], f32, kind="ExternalOutput")

    # round-robin tail runs into the down-phase gaps (after each dq except dq0)
    ngaps = len(passes) * (DQ - 1)
    gap_runs: dict = {}
    for r in range(nruns):
        gap_runs.setdefault(r % max(ngaps, 1), []).append(r)

    with tile.TileContext(nc) as tc:
        with (
            tc.tile_pool(name="xpool", bufs=1) as xpool,
            tc.tile_pool(name="hpool", bufs=1) as hpool,
            tc.tile_pool(name="wpool", bufs=2) as wpool,
            tc.tile_pool(name="w2pool", bufs=5) as w2pool,
            tc.tile_pool(name="spool", bufs=2) as spool,
            tc.tile_pool(name="opool", bufs=2) as opool,
            tc.tile_pool(name="tpool", bufs=1) as tpool,
            tc.tile_pool(name="htpool", bufs=2) as htpool,
            tc.tile_pool(name="w2tpool", bufs=2) as w2tpool,
            tc.tile_pool(name="const", bufs=1) as const_pool,
            tc.tile_pool(name="ps", bufs=8, space="PSUM") as ps_pool,
        ):
            # x resident for the whole kernel. First group + first weight tile
            # interleaved in 4-dc chunks so the first matmul starts early.
            xall = xpool.tile([P, DCH, base], bf16)
            wt0 = wpool.tile([P, DCH, 512], bf16, tag="wst")
            g0_t0, g0_sz = passes[0][0][0]
            for dc4 in range(0, DCH, 4):
                nc.sync.dma_start(
                    xall[:, dc4 : dc4 + 4, g0_t0 : g0_t0 + g0_sz],
                    xtp_d[:, dc4 : dc4 + 4, g0_t0 : g0_t0 + g0_sz],
                )
                nc.sync.dma_start(wt0[:, dc4 : dc4 + 4, :], wst_d[0, :, dc4 : dc4 + 4, :])
            wt1 = wpool.tile([P, DCH, 512], bf16, tag="wst")
            nc.sync.dma_start(wt1[:], wst_d[1])
            for gi, (groups, _) in enumerate(passes):
                for g, (t0, sz) in enumerate(groups):
                    if gi == 0 and g == 0:
                        continue
                    nc.sync.dma_start(xall[:, :, t0 : t0 + sz], xtp_d[:, :, t0 : t0 + sz])

            cw_sb = const_pool.tile([P, nsub], f32)
            nc.sync.dma_start(cw_sb[:], cw_d[:])
            if nruns:
                xt_sb = tpool.tile([P, DCH, TT], bf16)
                nc.sync.dma_start(xt_sb[:], xt_d[:])
                cwt_sb = const_pool.tile([P, nruns], f32)
                nc.sync.dma_start(cwt_sb[:], cwt_d[:])

            def tail_run(r, sz):
                """One TP-8 overflow run: gate/up over this core's I/8 shard
                (2 substeps of 256 i), then partial down (d in 4 tiles)."""
                o = run_off[r]
                hTt = htpool.tile([P, 4, sz], bf16, name=f"hTt_{r}")
                for s in range(2):
                    wtt = wpool.tile([P, DCH, 512], bf16, tag="wst", name=f"wstt_{r}_{s}")
                    nc.sync.dma_start(wtt[:], wst_t_d[r, s])
                    phg = [
                        ps_pool.tile([P, 512], f32, tag="ps", name=f"tphg_{r}_{s}_{j}")
                        for j in range(2)
                    ]
                    phu = [
                        ps_pool.tile([P, 512], f32, tag="ps", name=f"tphu_{r}_{s}_{j}")
                        for j in range(2)
                    ]
                    for dc in range(DCH):
                        for j in range(2):
                            nc.tensor.matmul(
                                phg[j][:, :sz],
                                wtt[:, dc, j * P : (j + 1) * P],
                                xt_sb[:, dc, o : o + sz],
                                start=(dc == 0),
                                stop=(dc == DCH - 1),
                            )
                            nc.tensor.matmul(
                                phu[j][:, :sz],
                                wtt[:, dc, 256 + j * P : 256 + (j + 1) * P],
                                xt_sb[:, dc, o : o + sz],
                                start=(dc == 0),
                                stop=(dc == DCH - 1),
                            )
                    for j in range(2):
                        sg = spool.tile([P, 512], f32, tag="sg")
                        nc.scalar.activation(
                            sg[:, :sz], phg[j][:, :sz], mybir.ActivationFunctionType.Silu
                        )
                        nc.vector.tensor_mul(
                            hTt[:, s * 2 + j, :], sg[:, :sz], phu[j][:, :sz]
                        )
                for half in range(2):
                    w2tt = w2tpool.tile([P, 4, 1024], bf16, tag="w2tt", name=f"w2tt_{r}_{half}")
                    nc.sync.dma_start(w2tt[:], w2t_t_d[r, half])
                    for dt in range(2):
                        po_t = ps_pool.tile([P, 512], f32, tag="ps", name=f"tpo_{r}_{half}_{dt}")
                        for ic in range(4):
                            nc.tensor.matmul(
                                po_t[:sz, :],
                                hTt[:, ic, :],
                                w2tt[:, ic, dt * 512 : (dt + 1) * 512],
                                start=(ic == 0),
                                stop=(ic == 3),
                            )
                        osb = opool.tile([P, 512], f32, tag="osb")
                        nc.scalar.activation(
                            osb[:sz, :],
                            po_t[:sz, :],
                            mybir.ActivationFunctionType.Copy,
                            scale=cwt_sb[:sz, r : r + 1],
                        )
                        dcol = (half * 2 + dt) * 512
                        nc.gpsimd.dma_start(
                            out=tout_d[o : o + sz, dcol : dcol + 512], in_=osb[:sz, :]
                        )

            # h^T for one macro-pass, bf16: [i-part, i-chunk, token]
            hT = hpool.tile([P, ICH, max_pass], bf16)
            sidx = 0
            gap = 0
            first_gateup = True
            for groups, subs in passes:
                pass_t0 = groups[0][0]

                # -- gate/up + SwiGLU; weights stream once over I per pass --
                for ig in range(IG):
                    if first_gateup and ig == 0:
                        wt = wt0
                    elif first_gateup and ig == 1:
                        wt = wt1
                    else:
                        wt = wpool.tile([P, DCH, 512], bf16, tag="wst")
                        nc.sync.dma_start(wt[:], wst_d[ig])
                    for t0, sz in groups:
                        phg = [
                            ps_pool.tile([P, 512], f32, tag="ps", name=f"phg_{t0}_{ig}_{j}")
                            for j in range(2)
                        ]
                        phu = [
                            ps_pool.tile([P, 512], f32, tag="ps", name=f"phu_{t0}_{ig}_{j}")
                            for j in range(2)
                        ]
                        for dc in range(DCH):
                            for j in range(2):
                                nc.tensor.matmul(
                                    phg[j][:, :sz],
                                    wt[:, dc, j * P : (j + 1) * P],
                                    xall[:, dc, t0 : t0 + sz],
                                    start=(dc == 0),
                                    stop=(dc == DCH - 1),
                                )
                                nc.tensor.matmul(
                                    phu[j][:, :sz],
                                    wt[:, dc, 256 + j * P : 256 + (j + 1) * P],
                                    xall[:, dc, t0 : t0 + sz],
                                    start=(dc == 0),
                                    stop=(dc == DCH - 1),
                                )
                        for j in range(2):
                            ic = ig * 2 + j
                            sg = spool.tile([P, 512], f32, tag="sg")
                            nc.scalar.activation(
                                sg[:, :sz], phg[j][:, :sz], mybir.ActivationFunctionType.Silu
                            )
                            nc.vector.tensor_mul(
                                hT[:, ic, t0 - pass_t0 : t0 - pass_t0 + sz],
                                sg[:, :sz],
                                phu[j][:, :sz],
                            )
                first_gateup = False

                # -- down proj; w2 streams once per pass, psum-accum over I.
                # Tail runs slot into the gaps after dq1..dq3 (DMA+PE slack). --
                for dq in range(DQ):
                    w2q = []
                    for q in range(4):
                        w2t = w2pool.tile([P, IQ, 512], bf16, tag="w2t")
                        nc.sync.dma_start(
                            w2t[:], w2t_d[dq, :, q * IQ : (q + 1) * IQ, :]
                        )
                        w2q.append(w2t)
                    for s, (t0, sz) in enumerate(subs):
                        r0 = t0 - pass_t0
                        po_t = ps_pool.tile([P, 512], f32, tag="ps", name=f"po_{t0}_{dq}")
                        for q in range(4):
                            for k in range(IQ):
                                ic = q * IQ + k
                                nc.tensor.matmul(
                                    po_t[:sz, :],
                                    hT[:, ic, r0 : r0 + sz],
                                    w2q[q][:, k, :],
                                    start=(ic == 0),
                                    stop=(ic == ICH - 1),
                                )
                        osb = opool.tile([P, 512], f32, tag="osb")
                        nc.scalar.activation(
                            osb[:sz, :],
                            po_t[:sz, :],
                            mybir.ActivationFunctionType.Copy,
                            scale=cw_sb[:sz, sidx + s : sidx + s + 1],
                        )
                        nc.gpsimd.dma_start(
                            out=out_d[t0 : t0 + sz, dq * 512 : (dq + 1) * 512],
                            in_=osb[:sz, :],
                        )
                    if dq > 0:
                        for r in gap_runs.pop(gap, []):
                            tail_run(r, run_sizes[r])
                        gap += 1
                sidx += len(subs)
            # any runs not placed (ngaps == 0 edge case)
            for rs in gap_runs.values():
                for r in rs:
                    tail_run(r, run_sizes[r])

    nc.compile()
    return nc


def _prepare(hidden_states, router_w, ws, w2s):
    x = np.asarray(hidden_states, dtype=np.float32).reshape(T, D)
    router_w = np.asarray(router_w, dtype=np.float32)
    ws = np.asarray(ws, dtype=np.float32)
    w2s = np.asarray(w2s, dtype=np.float32)

    top1, top2, w1, w2 = _host_router(x, router_w)

    toks: list[list[int]] = [[] for _ in range(E)]
    cws: list[list[float]] = [[] for _ in range(E)]
    for ti, wi in [(top1, w1), (top2, w2)]:
        for t in range(T):
            e = int(ti[t])
            toks[e].append(t)
            cws[e].append(float(wi[t]))

    n_max = max(max(len(tk) for tk in toks), 1)
    base = 128 * max(1, min(8, n_max // 128))

    # tail runs: per-expert overflow over `base`, split into <=128 chunks
    runs = []  # (expert, start_in_expert, size)
    for e in range(E):
        ov = len(toks[e]) - base
        q = base
        while ov > 0:
            sz = min(128, ov)
            runs.append((e, q, sz))
            q += sz
            ov -= sz
    run_sizes = tuple(sz for _, _, sz in runs)
    nruns = len(runs)
    TT = sum(run_sizes)
    nsub = sum(len(subs) for _, subs in _passes(base))

    xb = x.astype(BF16)
    gate_all = ws[:, :I, :]  # [E, I, D]
    up_all = ws[:, I:, :]

    # ---- shared (per-core-identical) tail inputs ----
    if nruns:
        tail_idx = np.concatenate(
            [np.asarray(toks[e][q : q + sz], dtype=np.int64) for e, q, sz in runs]
        )
        xt = np.ascontiguousarray(
            xb[tail_idx].reshape(TT, DCH, P).transpose(2, 1, 0)
        )  # [P, DCH, TT]
        cwt = np.zeros((nruns, P), dtype=np.float32)
        for r, (e, q, sz) in enumerate(runs):
            cwt[r, :sz] = np.asarray(cws[e][q : q + sz], dtype=np.float32)
        cwt = cwt.T.copy()  # [P, nruns]

    in_maps = []
    for c in range(E):
        n = len(toks[c])
        nb = min(n, base)
        idx = np.asarray(toks[c][:nb] + [0] * (base - nb), dtype=np.int64)
        xp = xb[idx]  # [base, D]
        if nb < base:
            xp[nb:] = 0
        xtp = np.ascontiguousarray(xp.reshape(base, DCH, P).transpose(2, 1, 0))

        cw_a = np.zeros((nsub * P,), dtype=np.float32)
        cw_a[:nb] = np.asarray(cws[c][:nb], dtype=np.float32)
        cw_a = cw_a.reshape(nsub, P).T.copy()  # [P, nsub]

        gate = gate_all[c]
        up = up_all[c]
        g_t = gate.reshape(IG, 256, DCH, P).transpose(0, 3, 2, 1)  # [IG, P, DCH, 256]
        u_t = up.reshape(IG, 256, DCH, P).transpose(0, 3, 2, 1)
        wst = np.ascontiguousarray(np.concatenate([g_t, u_t], axis=3)).astype(BF16)

        # w2t[dq, p, ic, j] = w2s[c][dq*512+j, ic*128+p]
        w2t = np.ascontiguousarray(
            w2s[c].reshape(DQ, 512, ICH, P).transpose(0, 3, 2, 1)
        ).astype(BF16)

        im = {"xtp": xtp, "wst": wst, "w2t": w2t, "cw": cw_a}

        if nruns:
            # tail weights: this core's I/8 shard of each run's expert
            lo = c * ISH
            wst_t = np.empty((nruns, 2, P, DCH, 512), dtype=BF16)
            w2t_t = np.empty((nruns, 2, P, 4, 1024), dtype=BF16)
            for r, (e, q, sz) in enumerate(runs):
                for s in range(2):
                    i0 = lo + s * 256
                    g_s = gate_all[e][i0 : i0 + 256].reshape(256, DCH, P).transpose(2, 1, 0)
                    u_s = up_all[e][i0 : i0 + 256].reshape(256, DCH, P).transpose(2, 1, 0)
                    wst_t[r, s] = np.concatenate([g_s, u_s], axis=2)  # [P, DCH, 512]
                # w2 shard: [P(i), ic(4), d]: w2t_t[r,h,p,ic,j] = w2s[e][h*1024+j, lo+ic*128+p]
                w2sh = (
                    w2s[e][:, lo : lo + ISH]
                    .reshape(2, 1024, 4, P)
                    .transpose(0, 3, 2, 1)
                )  # [2, P, 4, 1024]
                w2t_t[r] = w2sh.astype(BF16)
            im.update(
                {"xt_t": xt, "wst_t": wst_t, "w2t_t": w2t_t, "cwt": cwt}
            )
        in_maps.append(im)

    # output row mapping: main rows [c*base + pos], tail rows [8*base + off]
    pos = np.zeros((TOPK, T), dtype=np.int64)
    run_start = {}
    off = 0
    for r, (e, q, sz) in enumerate(runs):
        run_start[(e, q)] = off
        off += sz
    kidx = np.zeros(T, dtype=np.int64)
    for e in range(E):
        for q, t in enumerate(toks[e]):
            if q < base:
                row = e * base + q
            else:
                # find the run containing ordinal q
                rq = 128 * ((q - base) // 128) + base
                row = E * base + run_start[(e, rq)] + (q - rq)
            pos[kidx[t], t] = row
            kidx[t] += 1

    return base, run_sizes, pos, in_maps, TT


def kernel(hidden_states, router_w, ws, w2s):
    from concourse import bass_utils

    hs = np.asarray(hidden_states)
    B, S, _ = hs.shape
    base, run_sizes, pos, in_maps, TT = _prepare(hidden_states, router_w, ws, w2s)

    key = (base, run_sizes)
    if key not in _CACHE:
        _CACHE[key] = _build_bass(base, run_sizes)
    nc = _CACHE[key]

    res = bass_utils.run_bass_kernel_spmd(nc, in_maps, core_ids=list(range(NCORES)))
    main_rows = np.concatenate(
        [np.asarray(res.results[c]["out"], dtype=np.float32) for c in range(NCORES)],
        axis=0,
    )  # [8*base, D]
    if TT:
        tail_rows = np.sum(
            [np.asarray(res.results[c]["tout"], dtype=np.float32) for c in range(NCORES)],
            axis=0,
        )  # [TT, D]
        allrows = np.concatenate([main_rows, tail_rows], axis=0)
    else:
        allrows = main_rows
    out = allrows[pos[0]] + allrows[pos[1]]
    return out.reshape(B, S, D).astype(np.float32)


# revision 25
# speedup vs baseline: 1.0021x; 1.0021x over previous
"""DBRX MoE experts kernel for Trainium2 (8 NeuronCores).

Strategy (expert-parallel + TP-8 overflow tail, bf16):
  - Router (logits -> softmax -> top-2 -> renormalize) on host in numpy
    (0.01% of FLOPs); it determines the token->expert dispatch.
  - Main phase, expert-parallel: core c owns expert c (counts are near-
    balanced, ~1024 +- 40). Each core runs the full SwiGLU FFN for the first
    `base`=1024 tokens of its expert, scaling rows by the combine weight.
    No collective: the host adds the two expert contributions per token.
  - Overflow tail, tensor-parallel: the few tokens above `base` per expert
    (~90 total) are processed by ALL cores, each holding the I/8 shard of the
    run's expert weights; the host sums the 8 partial outputs. This removes
    the per-core padding to the max expert count: every core does identical
    work with zero load imbalance.
  - All matmuls bf16 (fp32 PSUM): full PE rate at any free dim, half the DMA
    and SBUF of fp32r. Main weights stream once per 512-token macro-pass
    (2 passes); x and h stay SBUF-resident. Tail runs' weight streams and
    compute are interleaved into the down-proj phases' DMA/PE slack.
"""

import numpy as np
import ml_dtypes

T = 4096
D = 2048
E = 8
I = 4096
TOPK = 2
NCORES = 8
P = 128
DCH = D // P  # 16 d-chunks (contraction tiles for gate/up)
IG = I // 256  # 16 i-groups of 256 (gate/up psum-pair granularity)
ICH = I // P  # 32 i-chunks (contraction tiles for down)
DQ = D // 512  # 4 output-column quarters for down
ISH = I // NCORES  # 512, tail i-shard

BF16 = ml_dtypes.bfloat16


def _host_router(x, router_w):
    """Replicate reference routing in numpy (fp32)."""
    logits = (x.astype(np.float64) @ router_w.astype(np.float64).T).astype(np.float32)
    m = logits.max(axis=-1, keepdims=True)
    ex = np.exp((logits - m).astype(np.float32))
    probs = ex / ex.sum(axis=-1, keepdims=True)
    # top-2, ties to lower index (matches jax.lax.top_k)
    top1 = probs.argmax(axis=-1)
    p = probs.copy()
    p[np.arange(T), top1] = -1.0
    top2 = p.argmax(axis=-1)
    w1 = probs[np.arange(T), top1]
    w2 = probs[np.arange(T), top2]
    s = w1 + w2
    return top1.astype(np.int64), top2.astype(np.int64), (w1 / s).astype(np.float32), (w2 / s).astype(np.float32)


def _ranges(t0, n, step):
    out = []
    t = t0
    while t0 + n - t > 0:
        sz = min(step, t0 + n - t)
        out.append((t, sz))
        t += sz
    return out


def _passes(cap):
    """Split [0, cap) into macro-passes of <=512 tokens (single gate/up psum
    group per pass)."""
    passes = []
    for t0, n in _ranges(0, cap, 512):
        groups = _ranges(t0, n, 512)
        subs = _ranges(t0, n, 128)
        passes.append((groups, subs))
    return passes


_CACHE: dict = {}


def _build_bass(base: int, run_sizes: tuple):
    """8-core SPMD program. Main: `base` token slots, expert-parallel.
    Tail: the given runs (<=128 tokens each), TP-8 over the I dim, partial
    outputs per core (host sums across cores)."""
    import concourse.bacc as bacc
    import concourse.mybir as mybir
    import concourse.tile as tile

    f32 = mybir.dt.float32
    bf16 = mybir.dt.bfloat16
    passes = _passes(base)
    nsub = sum(len(subs) for _, subs in passes)
    max_pass = max(g[-1][0] + g[-1][1] - g[0][0] for g, _ in passes)
    IQ = ICH // 4  # 8 i-chunks per main w2 quarter tile
    nruns = len(run_sizes)
    TT = sum(run_sizes)
    run_off = [sum(run_sizes[:r]) for r in range(nruns)]

    nc = bacc.Bacc("TRN2", target_bir_lowering=False)

    xtp_d = nc.dram_tensor("xtp", [P, DCH, base], bf16, kind="ExternalInput")
    wst_d = nc.dram_tensor("wst", [IG, P, DCH, 512], bf16, kind="ExternalInput")
    w2t_d = nc.dram_tensor("w2t", [DQ, P, ICH, 512], bf16, kind="ExternalInput")
    cw_d = nc.dram_tensor("cw", [P, nsub], f32, kind="ExternalInput")
    out_d = nc.dram_tensor("out", [base, D], f32, kind="ExternalOutput")
    if nruns:
        xt_d = nc.dram_tensor("xt_t", [P, DCH, TT], bf16, kind="ExternalInput")
        wst_t_d = nc.dram_tensor("wst_t", [nruns, 2, P, DCH, 512], bf16, kind="ExternalInput")
        w2t_t_d = nc.dram_tensor("w2t_t", [nruns, 2, P, 4, 1024], bf16, kind="ExternalInput")
        cwt_d = nc.dram_tensor("cwt", [P, nruns], f32, kind="ExternalInput")
        tout_d = nc.dram_tensor("tout", [TT, D], f32, kind="ExternalOutput")

    # round-robin tail runs into the down-phase gaps (after each dq except dq0)
    ngaps = len(passes) * (DQ - 1)
    gap_runs: dict = {}
    for r in range(nruns):
        gap_runs.setdefault(r % max(ngaps, 1), []).append(r)

    with tile.TileContext(nc) as tc:
        with (
            tc.tile_pool(name="xpool", bufs=1) as xpool,
            tc.tile_pool(name="hpool", bufs=1) as hpool,
            tc.tile_pool(name="wpool", bufs=2) as wpool,
            tc.tile_pool(name="w2pool", bufs=5) as w2pool,
            tc.tile_pool(name="spool", bufs=2) as spool,
            tc.tile_pool(name="opool", bufs=2) as opool,
            tc.tile_pool(name="tpool", bufs=1) as tpool,
            tc.tile_pool(name="htpool", bufs=2) as htpool,
            tc.tile_pool(name="w2tpool", bufs=2) as w2tpool,
            tc.tile_pool(name="const", bufs=1) as const_pool,
            tc.tile_pool(name="ps", bufs=8, space="PSUM") as ps_pool,
        ):
            # x resident for the whole kernel. First group + first weight tile
            # interleaved in 4-dc chunks so the first matmul starts early.
            xall = xpool.tile([P, DCH, base], bf16)
            wt0 = wpool.tile([P, DCH, 512], bf16, tag="wst")
            g0_t0, g0_sz = passes[0][0][0]
            for dc4 in range(0, DCH, 4):
                nc.sync.dma_start(
                    xall[:, dc4 : dc4 + 4, g0_t0 : g0_t0 + g0_sz],
                    xtp_d[:, dc4 : dc4 + 4, g0_t0 : g0_t0 + g0_sz],
                )
                nc.sync.dma_start(wt0[:, dc4 : dc4 + 4, :], wst_d[0, :, dc4 : dc4 + 4, :])
            wt1 = wpool.tile([P, DCH, 512], bf16, tag="wst")
            nc.sync.dma_start(wt1[:], wst_d[1])
            for gi, (groups, _) in enumerate(passes):
                for g, (t0, sz) in enumerate(groups):
                    if gi == 0 and g == 0:
                        continue
                    nc.sync.dma_start(xall[:, :, t0 : t0 + sz], xtp_d[:, :, t0 : t0 + sz])

            cw_sb = const_pool.tile([P, nsub], f32)
            nc.sync.dma_start(cw_sb[:], cw_d[:])
            if nruns:
                xt_sb = tpool.tile([P, DCH, TT], bf16)
                nc.sync.dma_start(xt_sb[:], xt_d[:])
                cwt_sb = const_pool.tile([P, nruns], f32)
                nc.sync.dma_start(cwt_sb[:], cwt_d[:])

            def tail_run(r, sz):
                """One TP-8 overflow run: gate/up over this core's I/8 shard
                (2 substeps of 256 i), then partial down (d in 4 tiles)."""
                o = run_off[r]
                hTt = htpool.tile([P, 4, sz], bf16, name=f"hTt_{r}")
                for s in range(2):
                    wtt = wpool.tile([P, DCH, 512], bf16, tag="wst", name=f"wstt_{r}_{s}")
                    nc.sync.dma_start(wtt[:], wst_t_d[r, s])
                    phg = [
                        ps_pool.tile([P, 512], f32, tag="ps", name=f"tphg_{r}_{s}_{j}")
                        for j in range(2)
                    ]
                    phu = [
                        ps_pool.tile([P, 512], f32, tag="ps", name=f"tphu_{r}_{s}_{j}")
                        for j in range(2)
                    ]
                    for dc in range(DCH):
                        for j in range(2):
                            nc.tensor.matmul(
                                phg[j][:, :sz],
                                wtt[:, dc, j * P : (j + 1) * P],
                                xt_sb[:, dc, o : o + sz],
                                start=(dc == 0),
                                stop=(dc == DCH - 1),
                            )
                            nc.tensor.matmul(
                                phu[j][:, :sz],
                                wtt[:, dc, 256 + j * P : 256 + (j + 1) * P],
                                xt_sb[:, dc, o : o + sz],
                                start=(dc == 0),
                                stop=(dc == DCH - 1),
                            )
                    for j in range(2):
                        sg = spool.tile([P, 512], f32, tag="sg")
                        nc.scalar.activation(
                            sg[:, :sz], phg[j][:, :sz], mybir.ActivationFunctionType.Silu
                        )
                        nc.vector.tensor_mul(
                            hTt[:, s * 2 + j, :], sg[:, :sz], phu[j][:, :sz]
                        )
                for half in range(2):
                    w2tt = w2tpool.tile([P, 4, 1024], bf16, tag="w2tt", name=f"w2tt_{r}_{half}")
                    nc.sync.dma_start(w2tt[:], w2t_t_d[r, half])
                    for dt in range(2):
                        po_t = ps_pool.tile([P, 512], f32, tag="ps", name=f"tpo_{r}_{half}_{dt}")
                        for ic in range(4):
                            nc.tensor.matmul(
                                po_t[:sz, :],
                                hTt[:, ic, :],
                                w2tt[:, ic, dt * 512 : (dt + 1) * 512],
                                start=(ic == 0),
                                stop=(ic == 3),
                            )
                        osb = opool.tile([P, 512], f32, tag="osb")
                        nc.scalar.activation(
                            osb[:sz, :],
                            po_t[:sz, :],
                            mybir.ActivationFunctionType.Copy,
                            scale=cwt_sb[:sz, r : r + 1],
                        )
                        dcol = (half * 2 + dt) * 512
                        nc.scalar.dma_start(
                            out=tout_d[o : o + sz, dcol : dcol + 512], in_=osb[:sz, :]
                        )

            # h^T for one macro-pass, bf16: [i-part, i-chunk, token]
            hT = hpool.tile([P, ICH, max_pass], bf16)
            sidx = 0
            gap = 0
            first_gateup = True
            for groups, subs in passes:
                pass_t0 = groups[0][0]

                # -- gate/up + SwiGLU; weights stream once over I per pass --
                for ig in range(IG):
                    if first_gateup and ig == 0:
                        wt = wt0
                    elif first_gateup and ig == 1:
                        wt = wt1
                    else:
                        wt = wpool.tile([P, DCH, 512], bf16, tag="wst")
                        nc.sync.dma_start(wt[:], wst_d[ig])
                    for t0, sz in groups:
                        phg = [
                            ps_pool.tile([P, 512], f32, tag="ps", name=f"phg_{t0}_{ig}_{j}")
                            for j in range(2)
                        ]
                        phu = [
                            ps_pool.tile([P, 512], f32, tag="ps", name=f"phu_{t0}_{ig}_{j}")
                            for j in range(2)
                        ]
                        for dc in range(DCH):
                            for j in range(2):
                                nc.tensor.matmul(
                                    phg[j][:, :sz],
                                    wt[:, dc, j * P : (j + 1) * P],
                                    xall[:, dc, t0 : t0 + sz],
                                    start=(dc == 0),
                                    stop=(dc == DCH - 1),
                                )
                                nc.tensor.matmul(
                                    phu[j][:, :sz],
                                    wt[:, dc, 256 + j * P : 256 + (j + 1) * P],
                                    xall[:, dc, t0 : t0 + sz],
                                    start=(dc == 0),
                                    stop=(dc == DCH - 1),
                                )
                        for j in range(2):
                            ic = ig * 2 + j
                            sg = spool.tile([P, 512], f32, tag="sg")
                            nc.scalar.activation(
                                sg[:, :sz], phg[j][:, :sz], mybir.ActivationFunctionType.Silu
                            )
                            nc.vector.tensor_mul(
                                hT[:, ic, t0 - pass_t0 : t0 - pass_t0 + sz],
                                sg[:, :sz],
                                phu[j][:, :sz],
                            )
                first_gateup = False

                # -- down proj; w2 streams once per pass, psum-accum over I.
                # Tail runs slot into the gaps after dq1..dq3 (DMA+PE slack). --
                for dq in range(DQ):
                    w2q = []
                    for q in range(4):
                        w2t = w2pool.tile([P, IQ, 512], bf16, tag="w2t")
                        nc.sync.dma_start(
                            w2t[:], w2t_d[dq, :, q * IQ : (q + 1) * IQ, :]
                        )
                        w2q.append(w2t)
                    for s, (t0, sz) in enumerate(subs):
                        r0 = t0 - pass_t0
                        po_t = ps_pool.tile([P, 512], f32, tag="ps", name=f"po_{t0}_{dq}")
                        for q in range(4):
                            for k in range(IQ):
                                ic = q * IQ + k
                                nc.tensor.matmul(
                                    po_t[:sz, :],
                                    hT[:, ic, r0 : r0 + sz],
                                    w2q[q][:, k, :],
                                    start=(ic == 0),
                                    stop=(ic == ICH - 1),
                                )
                        osb = opool.tile([P, 512], f32, tag="osb")
                        nc.scalar.activation(
                            osb[:sz, :],
                            po_t[:sz, :],
                            mybir.ActivationFunctionType.Copy,
                            scale=cw_sb[:sz, sidx + s : sidx + s + 1],
                        )
                        nc.scalar.dma_start(
                            out=out_d[t0 : t0 + sz, dq * 512 : (dq + 1) * 512],
                            in_=osb[:sz, :],
                        )
                    if dq > 0:
                        for r in gap_runs.pop(gap, []):
                            tail_run(r, run_sizes[r])
                        gap += 1
                sidx += len(subs)
            # any runs not placed (ngaps == 0 edge case)
            for rs in gap_runs.values():
                for r in rs:
                    tail_run(r, run_sizes[r])

    nc.compile()
    return nc


def _prepare(hidden_states, router_w, ws, w2s):
    x = np.asarray(hidden_states, dtype=np.float32).reshape(T, D)
    router_w = np.asarray(router_w, dtype=np.float32)
    ws = np.asarray(ws, dtype=np.float32)
    w2s = np.asarray(w2s, dtype=np.float32)

    top1, top2, w1, w2 = _host_router(x, router_w)

    toks: list[list[int]] = [[] for _ in range(E)]
    cws: list[list[float]] = [[] for _ in range(E)]
    for ti, wi in [(top1, w1), (top2, w2)]:
        for t in range(T):
            e = int(ti[t])
            toks[e].append(t)
            cws[e].append(float(wi[t]))

    n_max = max(max(len(tk) for tk in toks), 1)
    base = 128 * max(1, min(8, n_max // 128))

    # tail runs: per-expert overflow over `base`, split into <=128 chunks
    runs = []  # (expert, start_in_expert, size)
    for e in range(E):
        ov = len(toks[e]) - base
        q = base
        while ov > 0:
            sz = min(128, ov)
            runs.append((e, q, sz))
            q += sz
            ov -= sz
    run_sizes = tuple(sz for _, _, sz in runs)
    nruns = len(runs)
    TT = sum(run_sizes)
    nsub = sum(len(subs) for _, subs in _passes(base))

    xb = x.astype(BF16)
    gate_all = ws[:, :I, :]  # [E, I, D]
    up_all = ws[:, I:, :]

    # ---- shared (per-core-identical) tail inputs ----
    if nruns:
        tail_idx = np.concatenate(
            [np.asarray(toks[e][q : q + sz], dtype=np.int64) for e, q, sz in runs]
        )
        xt = np.ascontiguousarray(
            xb[tail_idx].reshape(TT, DCH, P).transpose(2, 1, 0)
        )  # [P, DCH, TT]
        cwt = np.zeros((nruns, P), dtype=np.float32)
        for r, (e, q, sz) in enumerate(runs):
            cwt[r, :sz] = np.asarray(cws[e][q : q + sz], dtype=np.float32)
        cwt = cwt.T.copy()  # [P, nruns]

    in_maps = []
    for c in range(E):
        n = len(toks[c])
        nb = min(n, base)
        idx = np.asarray(toks[c][:nb] + [0] * (base - nb), dtype=np.int64)
        xp = xb[idx]  # [base, D]
        if nb < base:
            xp[nb:] = 0
        xtp = np.ascontiguousarray(xp.reshape(base, DCH, P).transpose(2, 1, 0))

        cw_a = np.zeros((nsub * P,), dtype=np.float32)
        cw_a[:nb] = np.asarray(cws[c][:nb], dtype=np.float32)
        cw_a = cw_a.reshape(nsub, P).T.copy()  # [P, nsub]

        gate = gate_all[c]
        up = up_all[c]
        g_t = gate.reshape(IG, 256, DCH, P).transpose(0, 3, 2, 1)  # [IG, P, DCH, 256]
        u_t = up.reshape(IG, 256, DCH, P).transpose(0, 3, 2, 1)
        wst = np.ascontiguousarray(np.concatenate([g_t, u_t], axis=3)).astype(BF16)

        # w2t[dq, p, ic, j] = w2s[c][dq*512+j, ic*128+p]
        w2t = np.ascontiguousarray(
            w2s[c].reshape(DQ, 512, ICH, P).transpose(0, 3, 2, 1)
        ).astype(BF16)

        im = {"xtp": xtp, "wst": wst, "w2t": w2t, "cw": cw_a}

        if nruns:
            # tail weights: this core's I/8 shard of each run's expert
            lo = c * ISH
            wst_t = np.empty((nruns, 2, P, DCH, 512), dtype=BF16)
            w2t_t = np.empty((nruns, 2, P, 4, 1024), dtype=BF16)
            for r, (e, q, sz) in enumerate(runs):
                for s in range(2):
                    i0 = lo + s * 256
                    g_s = gate_all[e][i0 : i0 + 256].reshape(256, DCH, P).transpose(2, 1, 0)
                    u_s = up_all[e][i0 : i0 + 256].reshape(256, DCH, P).transpose(2, 1, 0)
                    wst_t[r, s] = np.concatenate([g_s, u_s], axis=2)  # [P, DCH, 512]
                # w2 shard: [P(i), ic(4), d]: w2t_t[r,h,p,ic,j] = w2s[e][h*1024+j, lo+ic*128+p]
                w2sh = (
                    w2s[e][:, lo : lo + ISH]
                    .reshape(2, 1024, 4, P)
                    .transpose(0, 3, 2, 1)
                )  # [2, P, 4, 1024]
                w2t_t[r] = w2sh.astype(BF16)
            im.update(
                {"xt_t": xt, "wst_t": wst_t, "w2t_t": w2t_t, "cwt": cwt}
            )
        in_maps.append(im)

    # output row mapping: main rows [c*base + pos], tail rows [8*base + off]
    pos = np.zeros((TOPK, T), dtype=np.int64)
    run_start = {}
    off = 0
    for r, (e, q, sz) in enumerate(runs):
        run_start[(e, q)] = off
        off += sz
    kidx = np.zeros(T, dtype=np.int64)
    for e in range(E):
        for q, t in enumerate(toks[e]):
            if q < base:
                row = e * base + q
            else:
                # find the run containing ordinal q
                rq = 128 * ((q - base) // 128) + base
                row = E * base + run_start[(e, rq)] + (q - rq)
            pos[kidx[t], t] = row
            kidx[t] += 1

    return base, run_sizes, pos, in_maps, TT


def kernel(hidden_states, router_w, ws, w2s):
    from concourse import bass_utils

    hs = np.asarray(hidden_states)
    B, S, _ = hs.shape
    base, run_sizes, pos, in_maps, TT = _prepare(hidden_states, router_w, ws, w2s)

    key = (base, run_sizes)
    if key not in _CACHE:
        _CACHE[key] = _build_bass(base, run_sizes)
    nc = _CACHE[key]

    res = bass_utils.run_bass_kernel_spmd(nc, in_maps, core_ids=list(range(NCORES)))
    main_rows = np.concatenate(
        [np.asarray(res.results[c]["out"], dtype=np.float32) for c in range(NCORES)],
        axis=0,
    )  # [8*base, D]
    if TT:
        tail_rows = np.sum(
            [np.asarray(res.results[c]["tout"], dtype=np.float32) for c in range(NCORES)],
            axis=0,
        )  # [TT, D]
        allrows = np.concatenate([main_rows, tail_rows], axis=0)
    else:
        allrows = main_rows
    out = allrows[pos[0]] + allrows[pos[1]]
    return out.reshape(B, S, D).astype(np.float32)


# revision 36
# speedup vs baseline: 1.0646x; 1.0623x over previous
"""DBRX MoE experts kernel for Trainium2 (8 NeuronCores).

Strategy (expert-parallel + TP-8 overflow tail, bf16):
  - Router (logits -> softmax -> top-2 -> renormalize) on host in numpy
    (0.01% of FLOPs); it determines the token->expert dispatch.
  - Main phase, expert-parallel: core c owns expert c (counts are near-
    balanced, ~1024 +- 40). Each core runs the full SwiGLU FFN for the first
    `base`=1024 tokens of its expert, scaling rows by the combine weight.
    No collective: the host adds the two expert contributions per token.
  - Overflow tail, tensor-parallel: the few tokens above `base` per expert
    (~90 total) are processed by ALL cores, each holding the I/8 shard of the
    run's expert weights; the host sums the 8 partial outputs. This removes
    the per-core padding to the max expert count: every core does identical
    work with zero load imbalance.
  - All matmuls bf16 (fp32 PSUM): full PE rate at any free dim, half the DMA
    and SBUF of fp32r. Main weights stream once per 512-token macro-pass
    (2 passes); x and h stay SBUF-resident. Tail runs' weight streams and
    compute are interleaved into the down-proj phases' DMA/PE slack.
"""

import numpy as np
import ml_dtypes

T = 4096
D = 2048
E = 8
I = 4096
TOPK = 2
NCORES = 8
P = 128
DCH = D // P  # 16 d-chunks (contraction tiles for gate/up)
IG = I // 256  # 16 i-groups of 256 (gate/up psum-pair granularity)
ICH = I // P  # 32 i-chunks (contraction tiles for down)
DQ = D // 512  # 4 output-column quarters for down
ISH = I // NCORES  # 512, tail i-shard

BF16 = ml_dtypes.bfloat16


def _host_router(x, router_w):
    """Replicate reference routing in numpy (fp32)."""
    logits = (x.astype(np.float64) @ router_w.astype(np.float64).T).astype(np.float32)
    m = logits.max(axis=-1, keepdims=True)
    ex = np.exp((logits - m).astype(np.float32))
    probs = ex / ex.sum(axis=-1, keepdims=True)
    # top-2, ties to lower index (matches jax.lax.top_k)
    top1 = probs.argmax(axis=-1)
    p = probs.copy()
    p[np.arange(T), top1] = -1.0
    top2 = p.argmax(axis=-1)
    w1 = probs[np.arange(T), top1]
    w2 = probs[np.arange(T), top2]
    s = w1 + w2
    return top1.astype(np.int64), top2.astype(np.int64), (w1 / s).astype(np.float32), (w2 / s).astype(np.float32)


def _ranges(t0, n, step):
    out = []
    t = t0
    while t0 + n - t > 0:
        sz = min(step, t0 + n - t)
        out.append((t, sz))
        t += sz
    return out


def _passes(cap):
    """Split [0, cap) into macro-passes of <=512 tokens (single gate/up psum
    group per pass)."""
    passes = []
    for t0, n in _ranges(0, cap, 512):
        groups = _ranges(t0, n, 512)
        subs = _ranges(t0, n, 128)
        passes.append((groups, subs))
    return passes


_CACHE: dict = {}


def _build_bass(base: int, run_sizes: tuple):
    """8-core SPMD program. Main: `base` token slots, expert-parallel.
    Tail: the given runs (<=128 tokens each), TP-8 over the I dim, partial
    outputs per core (host sums across cores)."""
    import concourse.bacc as bacc
    import concourse.mybir as mybir
    import concourse.tile as tile

    f32 = mybir.dt.float32
    bf16 = mybir.dt.bfloat16
    passes = _passes(base)
    nsub = sum(len(subs) for _, subs in passes)
    max_pass = max(g[-1][0] + g[-1][1] - g[0][0] for g, _ in passes)
    IQ = ICH // 4  # 8 i-chunks per main w2 quarter tile
    nruns = len(run_sizes)
    TT = sum(run_sizes)
    run_off = [sum(run_sizes[:r]) for r in range(nruns)]

    nc = bacc.Bacc("TRN2", target_bir_lowering=False)

    xtp_d = nc.dram_tensor("xtp", [P, DCH, base], bf16, kind="ExternalInput")
    wst_d = nc.dram_tensor("wst", [IG, P, DCH, 512], bf16, kind="ExternalInput")
    w2t_d = nc.dram_tensor("w2t", [DQ, P, ICH, 512], bf16, kind="ExternalInput")
    cw_d = nc.dram_tensor("cw", [P, nsub], f32, kind="ExternalInput")
    out_d = nc.dram_tensor("out", [base, D], f32, kind="ExternalOutput")
    if nruns:
        xt_d = nc.dram_tensor("xt_t", [P, DCH, TT], bf16, kind="ExternalInput")
        wst_t_d = nc.dram_tensor("wst_t", [nruns, 2, P, DCH, 512], bf16, kind="ExternalInput")
        w2t_t_d = nc.dram_tensor("w2t_t", [nruns, 2, P, 4, 1024], bf16, kind="ExternalInput")
        cwt_d = nc.dram_tensor("cwt", [P, nruns], f32, kind="ExternalInput")
        tout_d = nc.dram_tensor("tout", [TT, D], f32, kind="ExternalOutput")

    with tile.TileContext(nc) as tc:
        with (
            tc.tile_pool(name="xpool", bufs=1) as xpool,
            tc.tile_pool(name="hpool", bufs=1) as hpool,
            tc.tile_pool(name="wpool", bufs=2) as wpool,
            tc.tile_pool(name="w2pool", bufs=5) as w2pool,
            tc.tile_pool(name="spool", bufs=2) as spool,
            tc.tile_pool(name="opool", bufs=2) as opool,
            tc.tile_pool(name="tpool", bufs=1) as tpool,
            tc.tile_pool(name="htpool", bufs=2) as htpool,
            tc.tile_pool(name="w2tpool", bufs=2) as w2tpool,
            tc.tile_pool(name="const", bufs=1) as const_pool,
            tc.tile_pool(name="ps", bufs=8, space="PSUM") as ps_pool,
        ):
            # x resident for the whole kernel. First group + first weight tile
            # interleaved in 4-dc chunks so the first matmul starts early.
            xall = xpool.tile([P, DCH, base], bf16)
            wt0 = wpool.tile([P, DCH, 512], bf16, tag="wst")
            g0_t0, g0_sz = passes[0][0][0]
            for dc4 in range(0, DCH, 4):
                nc.sync.dma_start(
                    xall[:, dc4 : dc4 + 4, g0_t0 : g0_t0 + g0_sz],
                    xtp_d[:, dc4 : dc4 + 4, g0_t0 : g0_t0 + g0_sz],
                )
                nc.sync.dma_start(wt0[:, dc4 : dc4 + 4, :], wst_d[0, :, dc4 : dc4 + 4, :])
            wt1 = wpool.tile([P, DCH, 512], bf16, tag="wst")
            nc.sync.dma_start(wt1[:], wst_d[1])
            for gi, (groups, _) in enumerate(passes):
                for g, (t0, sz) in enumerate(groups):
                    if gi == 0 and g == 0:
                        continue
                    nc.sync.dma_start(xall[:, :, t0 : t0 + sz], xtp_d[:, :, t0 : t0 + sz])

            cw_sb = const_pool.tile([P, nsub], f32)
            nc.sync.dma_start(cw_sb[:], cw_d[:])
            if nruns:
                xt_sb = tpool.tile([P, DCH, TT], bf16)
                nc.sync.dma_start(xt_sb[:], xt_d[:])
                cwt_sb = const_pool.tile([P, nruns], f32)
                nc.sync.dma_start(cwt_sb[:], cwt_d[:])

            def tail_run_stages(r, sz):
                """One TP-8 overflow run as 4 stages (2 gate/up substeps of
                256 i each, 2 down halves), to be interleaved between main
                down subblocks so main compute hides the tail's DMA/ACT/DVE
                latencies."""
                o = run_off[r]
                ht_cell: list = []

                def gu_sub(s):
                    if s == 0:
                        ht_cell.append(htpool.tile([P, 4, sz], bf16, name=f"hTt_{r}"))
                    hTt = ht_cell[0]
                    wtt = wpool.tile([P, DCH, 512], bf16, tag="wst", name=f"wstt_{r}_{s}")
                    nc.sync.dma_start(wtt[:], wst_t_d[r, s])
                    phg = [
                        ps_pool.tile([P, 512], f32, tag="ps", name=f"tphg_{r}_{s}_{j}")
                        for j in range(2)
                    ]
                    phu = [
                        ps_pool.tile([P, 512], f32, tag="ps", name=f"tphu_{r}_{s}_{j}")
                        for j in range(2)
                    ]
                    for dc in range(DCH):
                        for j in range(2):
                            nc.tensor.matmul(
                                phg[j][:, :sz],
                                wtt[:, dc, j * P : (j + 1) * P],
                                xt_sb[:, dc, o : o + sz],
                                start=(dc == 0),
                                stop=(dc == DCH - 1),
                            )
                            nc.tensor.matmul(
                                phu[j][:, :sz],
                                wtt[:, dc, 256 + j * P : 256 + (j + 1) * P],
                                xt_sb[:, dc, o : o + sz],
                                start=(dc == 0),
                                stop=(dc == DCH - 1),
                            )
                    for j in range(2):
                        sg = spool.tile([P, 512], f32, tag="sg")
                        nc.scalar.activation(
                            sg[:, :sz], phg[j][:, :sz], mybir.ActivationFunctionType.Silu
                        )
                        nc.vector.tensor_mul(
                            hTt[:, s * 2 + j, :], sg[:, :sz], phu[j][:, :sz]
                        )
                def down_half(half):
                    hTt = ht_cell[0]
                    w2tt = w2tpool.tile([P, 4, 1024], bf16, tag="w2tt", name=f"w2tt_{r}_{half}")
                    nc.sync.dma_start(w2tt[:], w2t_t_d[r, half])
                    for dt in range(2):
                        po_t = ps_pool.tile([P, 512], f32, tag="ps", name=f"tpo_{r}_{half}_{dt}")
                        for ic in range(4):
                            nc.tensor.matmul(
                                po_t[:sz, :],
                                hTt[:, ic, :],
                                w2tt[:, ic, dt * 512 : (dt + 1) * 512],
                                start=(ic == 0),
                                stop=(ic == 3),
                            )
                        osb = opool.tile([P, 512], f32, tag="osb")
                        nc.scalar.activation(
                            osb[:sz, :],
                            po_t[:sz, :],
                            mybir.ActivationFunctionType.Copy,
                            scale=cwt_sb[:sz, r : r + 1],
                        )
                        dcol = (half * 2 + dt) * 512
                        nc.sync.dma_start(
                            out=tout_d[o : o + sz, dcol : dcol + 512], in_=osb[:sz, :]
                        )

                return [
                    lambda: gu_sub(0),
                    lambda: gu_sub(1),
                    lambda: down_half(0),
                    lambda: down_half(1),
                ]

            # h^T for one macro-pass, bf16: [i-part, i-chunk, token]
            hT = hpool.tile([P, ICH, max_pass], bf16)
            # tail stages, one emitted after each main down subblock (from
            # dq1 on) so main PE work hides each stage's DMA/ACT/DVE latency
            stages: list = []
            for r in range(nruns):
                stages.extend(tail_run_stages(r, run_sizes[r]))
            sidx = 0
            first_gateup = True
            for groups, subs in passes:
                pass_t0 = groups[0][0]

                # -- gate/up + SwiGLU; weights stream once over I per pass --
                for ig in range(IG):
                    if first_gateup and ig == 0:
                        wt = wt0
                    elif first_gateup and ig == 1:
                        wt = wt1
                    else:
                        wt = wpool.tile([P, DCH, 512], bf16, tag="wst")
                        nc.sync.dma_start(wt[:], wst_d[ig])
                    for t0, sz in groups:
                        phg = [
                            ps_pool.tile([P, 512], f32, tag="ps", name=f"phg_{t0}_{ig}_{j}")
                            for j in range(2)
                        ]
                        phu = [
                            ps_pool.tile([P, 512], f32, tag="ps", name=f"phu_{t0}_{ig}_{j}")
                            for j in range(2)
                        ]
                        for dc in range(DCH):
                            for j in range(2):
                                nc.tensor.matmul(
                                    phg[j][:, :sz],
                                    wt[:, dc, j * P : (j + 1) * P],
                                    xall[:, dc, t0 : t0 + sz],
                                    start=(dc == 0),
                                    stop=(dc == DCH - 1),
                                )
                                nc.tensor.matmul(
                                    phu[j][:, :sz],
                                    wt[:, dc, 256 + j * P : 256 + (j + 1) * P],
                                    xall[:, dc, t0 : t0 + sz],
                                    start=(dc == 0),
                                    stop=(dc == DCH - 1),
                                )
                        for j in range(2):
                            ic = ig * 2 + j
                            sg = spool.tile([P, 512], f32, tag="sg")
                            nc.scalar.activation(
                                sg[:, :sz], phg[j][:, :sz], mybir.ActivationFunctionType.Silu
                            )
                            nc.vector.tensor_mul(
                                hT[:, ic, t0 - pass_t0 : t0 - pass_t0 + sz],
                                sg[:, :sz],
                                phu[j][:, :sz],
                            )
                first_gateup = False

                # -- down proj; w2 streams once per pass, psum-accum over I.
                # Tail run stages slot in after each subblock (DMA+PE slack). --
                for dq in range(DQ):
                    w2q = []
                    for q in range(4):
                        w2t = w2pool.tile([P, IQ, 512], bf16, tag="w2t")
                        nc.sync.dma_start(
                            w2t[:], w2t_d[dq, :, q * IQ : (q + 1) * IQ, :]
                        )
                        w2q.append(w2t)
                    for s, (t0, sz) in enumerate(subs):
                        r0 = t0 - pass_t0
                        po_t = ps_pool.tile([P, 512], f32, tag="ps", name=f"po_{t0}_{dq}")
                        for q in range(4):
                            for k in range(IQ):
                                ic = q * IQ + k
                                nc.tensor.matmul(
                                    po_t[:sz, :],
                                    hT[:, ic, r0 : r0 + sz],
                                    w2q[q][:, k, :],
                                    start=(ic == 0),
                                    stop=(ic == ICH - 1),
                                )
                        osb = opool.tile([P, 512], f32, tag="osb")
                        nc.scalar.activation(
                            osb[:sz, :],
                            po_t[:sz, :],
                            mybir.ActivationFunctionType.Copy,
                            scale=cw_sb[:sz, sidx + s : sidx + s + 1],
                        )
                        nc.sync.dma_start(
                            out=out_d[t0 : t0 + sz, dq * 512 : (dq + 1) * 512],
                            in_=osb[:sz, :],
                        )
                        if dq > 0 and stages:
                            stages.pop(0)()
                sidx += len(subs)
            # any stages not yet emitted
            for st in stages:
                st()

    nc.compile()
    return nc


def _prepare(hidden_states, router_w, ws, w2s):
    x = np.asarray(hidden_states, dtype=np.float32).reshape(T, D)
    router_w = np.asarray(router_w, dtype=np.float32)
    ws = np.asarray(ws, dtype=np.float32)
    w2s = np.asarray(w2s, dtype=np.float32)

    top1, top2, w1, w2 = _host_router(x, router_w)

    toks: list[list[int]] = [[] for _ in range(E)]
    cws: list[list[float]] = [[] for _ in range(E)]
    for ti, wi in [(top1, w1), (top2, w2)]:
        for t in range(T):
            e = int(ti[t])
            toks[e].append(t)
            cws[e].append(float(wi[t]))

    n_max = max(max(len(tk) for tk in toks), 1)
    base = 128 * max(1, min(8, n_max // 128))

    # tail runs: per-expert overflow over `base`, split into <=128 chunks
    runs = []  # (expert, start_in_expert, size)
    for e in range(E):
        ov = len(toks[e]) - base
        q = base
        while ov > 0:
            sz = min(128, ov)
            runs.append((e, q, sz))
            q += sz
            ov -= sz
    run_sizes = tuple(sz for _, _, sz in runs)
    nruns = len(runs)
    TT = sum(run_sizes)
    nsub = sum(len(subs) for _, subs in _passes(base))

    xb = x.astype(BF16)
    gate_all = ws[:, :I, :]  # [E, I, D]
    up_all = ws[:, I:, :]

    # ---- shared (per-core-identical) tail inputs ----
    if nruns:
        tail_idx = np.concatenate(
            [np.asarray(toks[e][q : q + sz], dtype=np.int64) for e, q, sz in runs]
        )
        xt = np.ascontiguousarray(
            xb[tail_idx].reshape(TT, DCH, P).transpose(2, 1, 0)
        )  # [P, DCH, TT]
        cwt = np.zeros((nruns, P), dtype=np.float32)
        for r, (e, q, sz) in enumerate(runs):
            cwt[r, :sz] = np.asarray(cws[e][q : q + sz], dtype=np.float32)
        cwt = cwt.T.copy()  # [P, nruns]

    in_maps = []
    for c in range(E):
        n = len(toks[c])
        nb = min(n, base)
        idx = np.asarray(toks[c][:nb] + [0] * (base - nb), dtype=np.int64)
        xp = xb[idx]  # [base, D]
        if nb < base:
            xp[nb:] = 0
        xtp = np.ascontiguousarray(xp.reshape(base, DCH, P).transpose(2, 1, 0))

        cw_a = np.zeros((nsub * P,), dtype=np.float32)
        cw_a[:nb] = np.asarray(cws[c][:nb], dtype=np.float32)
        cw_a = cw_a.reshape(nsub, P).T.copy()  # [P, nsub]

        gate = gate_all[c]
        up = up_all[c]
        g_t = gate.reshape(IG, 256, DCH, P).transpose(0, 3, 2, 1)  # [IG, P, DCH, 256]
        u_t = up.reshape(IG, 256, DCH, P).transpose(0, 3, 2, 1)
        wst = np.ascontiguousarray(np.concatenate([g_t, u_t], axis=3)).astype(BF16)

        # w2t[dq, p, ic, j] = w2s[c][dq*512+j, ic*128+p]
        w2t = np.ascontiguousarray(
            w2s[c].reshape(DQ, 512, ICH, P).transpose(0, 3, 2, 1)
        ).astype(BF16)

        im = {"xtp": xtp, "wst": wst, "w2t": w2t, "cw": cw_a}

        if nruns:
            # tail weights: this core's I/8 shard of each run's expert
            lo = c * ISH
            wst_t = np.empty((nruns, 2, P, DCH, 512), dtype=BF16)
            w2t_t = np.empty((nruns, 2, P, 4, 1024), dtype=BF16)
            for r, (e, q, sz) in enumerate(runs):
                for s in range(2):
                    i0 = lo + s * 256
                    g_s = gate_all[e][i0 : i0 + 256].reshape(256, DCH, P).transpose(2, 1, 0)
                    u_s = up_all[e][i0 : i0 + 256].reshape(256, DCH, P).transpose(2, 1, 0)
                    wst_t[r, s] = np.concatenate([g_s, u_s], axis=2)  # [P, DCH, 512]
                # w2 shard: [P(i), ic(4), d]: w2t_t[r,h,p,ic,j] = w2s[e][h*1024+j, lo+ic*128+p]
                w2sh = (
                    w2s[e][:, lo : lo + ISH]
                    .reshape(2, 1024, 4, P)
                    .transpose(0, 3, 2, 1)
                )  # [2, P, 4, 1024]
                w2t_t[r] = w2sh.astype(BF16)
            im.update(
                {"xt_t": xt, "wst_t": wst_t, "w2t_t": w2t_t, "cwt": cwt}
            )
        in_maps.append(im)

    # output row mapping: main rows [c*base + pos], tail rows [8*base + off]
    pos = np.zeros((TOPK, T), dtype=np.int64)
    run_start = {}
    off = 0
    for r, (e, q, sz) in enumerate(runs):
        run_start[(e, q)] = off
        off += sz
    kidx = np.zeros(T, dtype=np.int64)
    for e in range(E):
        for q, t in enumerate(toks[e]):
            if q < base:
                row = e * base + q
            else:
                # find the run containing ordinal q
                rq = 128 * ((q - base) // 128) + base
                row = E * base + run_start[(e, rq)] + (q - rq)
            pos[kidx[t], t] = row
            kidx[t] += 1

    return base, run_sizes, pos, in_maps, TT


def kernel(hidden_states, router_w, ws, w2s):
    from concourse import bass_utils

    hs = np.asarray(hidden_states)
    B, S, _ = hs.shape
    base, run_sizes, pos, in_maps, TT = _prepare(hidden_states, router_w, ws, w2s)

    key = (base, run_sizes)
    if key not in _CACHE:
        _CACHE[key] = _build_bass(base, run_sizes)
    nc = _CACHE[key]

    res = bass_utils.run_bass_kernel_spmd(nc, in_maps, core_ids=list(range(NCORES)))
    main_rows = np.concatenate(
        [np.asarray(res.results[c]["out"], dtype=np.float32) for c in range(NCORES)],
        axis=0,
    )  # [8*base, D]
    if TT:
        tail_rows = np.sum(
            [np.asarray(res.results[c]["tout"], dtype=np.float32) for c in range(NCORES)],
            axis=0,
        )  # [TT, D]
        allrows = np.concatenate([main_rows, tail_rows], axis=0)
    else:
        allrows = main_rows
    out = allrows[pos[0]] + allrows[pos[1]]
    return out.reshape(B, S, D).astype(np.float32)
